# revision 3
# baseline (speedup 1.0000x reference)
"""Top-1 MoE layer (Mistral MLP experts, E=2) on 8 Trainium2 cores.

Strategy (expert-parallel + data-parallel, host does dispatch/combine):
  - Host computes the tiny router (T x E logits, softmax, argmax) in fp64,
    sorts token indices by assigned expert, and splits each expert's tokens
    evenly across that expert's cores (4 cores per expert when balanced).
  - Each core receives: its packed tokens (transposed, bf16, k-tiled), its
    expert's weights pre-tiled to make every device DMA fully contiguous,
    and the routing weight per token (replicated across partitions).
  - Device kernel per core: gate/up matmuls (bf16, fp32 PSUM accum),
    silu(g)*u fused on ACT+DVE, down matmul, per-token routing-weight scale
    fused into the PSUM->SBUF copy. No collectives needed.
  - Host scatters per-core outputs back to token order.
"""

import math

import numpy as np
import ml_dtypes

B, S, D, FF, E = 4, 2048, 2048, 8192, 2
T = B * S
P = 128
TC = 512  # token chunk = matmul free dim
KT = D // P  # 16 contraction tiles for gate/up
FT = FF // P  # 64 f tiles
DT = D // P  # 16 output-row tiles for down
N_CORES = 8

_nc_cache: dict[int, object] = {}

# Last BassKernelResults (for external profiling harnesses).
LAST = None


def _build_nc(C: int):
    """Build + compile the single-core Bass program (SPMD across 8 cores).

    C = per-core token capacity (multiple of 128).
    """
    import concourse.mybir as mybir
    import concourse.tile as tile
    from concourse import bacc

    dt = mybir.dt
    nc = bacc.Bacc("TRN2", target_bir_lowering=False, debug=False,
                   num_devices=N_CORES)

    # xt[p, ki, t] = x_packed[t, ki*128 + p]
    xt_d = nc.dram_tensor("xt", [P, KT, C], dt.bfloat16, kind="ExternalInput")
    # wg[f, p, ki, m] = w_gate[f*128+m, ki*128+p] (one expert)
    wg_d = nc.dram_tensor("wg", [FT, P, KT, P], dt.bfloat16, kind="ExternalInput")
    wu_d = nc.dram_tensor("wu", [FT, P, KT, P], dt.bfloat16, kind="ExternalInput")
    # wd[do, p, f, m] = w_down[do*128+m, f*128+p]
    wd_d = nc.dram_tensor("wd", [DT, P, FT, P], dt.bfloat16, kind="ExternalInput")
    # tw[p, t] = routing weight of token t (same for all p)
    tw_d = nc.dram_tensor("tw", [P, C], dt.float32, kind="ExternalInput")
    # y[do, m, t] = out_packed[t, do*128+m]
    y_d = nc.dram_tensor("y", [DT, P, C], dt.float32, kind="ExternalOutput")

    n_chunks = (C + TC - 1) // TC

    with tile.TileContext(nc) as tc:
        with (
            tc.tile_pool(name="persist", bufs=1) as pp,
            tc.tile_pool(name="wgwu", bufs=3) as wp,
            tc.tile_pool(name="wd", bufs=2) as dp,
            tc.tile_pool(name="hbuf", bufs=1) as hp,
            tc.tile_pool(name="stage", bufs=2) as sp,
            tc.tile_pool(name="psum", bufs=2, space="PSUM") as psp,
        ):
            xt = pp.tile([P, KT, C], dt.bfloat16)
            nc.sync.dma_start(out=xt[:], in_=xt_d[:])
            tw = pp.tile([P, C], dt.float32)
            nc.sync.dma_start(out=tw[:], in_=tw_d[:])
            h = hp.tile([P, FT, TC], dt.bfloat16)

            for c in range(n_chunks):
                tn = min(TC, C - c * TC)
                tsl = slice(c * TC, c * TC + tn)
                # phase A: h = silu(x @ Wg^T) * (x @ Wu^T), f-tile by f-tile
                for f in range(FT):
                    wg_t = wp.tile([P, KT, P], dt.bfloat16, tag="wg")
                    nc.sync.dma_start(out=wg_t[:], in_=wg_d[f])
                    wu_t = wp.tile([P, KT, P], dt.bfloat16, tag="wu")
                    nc.sync.dma_start(out=wu_t[:], in_=wu_d[f])
                    g_ps = psp.tile([P, TC], dt.float32, tag="g")
                    u_ps = psp.tile([P, TC], dt.float32, tag="u")
                    for ki in range(KT):
                        nc.tensor.matmul(
                            g_ps[:, :tn],
                            wg_t[:, ki : ki + 1, :],
                            xt[:, ki : ki + 1, tsl],
                            start=(ki == 0),
                            stop=(ki == KT - 1),
                        )
                    for ki in range(KT):
                        nc.tensor.matmul(
                            u_ps[:, :tn],
                            wu_t[:, ki : ki + 1, :],
                            xt[:, ki : ki + 1, tsl],
                            start=(ki == 0),
                            stop=(ki == KT - 1),
                        )
                    sg = sp.tile([P, TC], dt.float32, tag="sg")
                    nc.scalar.activation(
                        sg[:, :tn], g_ps[:, :tn],
                        mybir.ActivationFunctionType.Silu,
                    )
                    nc.vector.tensor_mul(h[:, f, :tn], sg[:, :tn], u_ps[:, :tn])
                # phase B: y = (h @ Wd^T) * tw
                for do in range(DT):
                    wd_t = dp.tile([P, FT, P], dt.bfloat16, tag="wd")
                    nc.sync.dma_start(out=wd_t[:], in_=wd_d[do])
                    y_ps = psp.tile([P, TC], dt.float32, tag="y")
                    for f in range(FT):
                        nc.tensor.matmul(
                            y_ps[:, :tn],
                            wd_t[:, f : f + 1, :],
                            h[:, f : f + 1, :tn],
                            start=(f == 0),
                            stop=(f == FT - 1),
                        )
                    y_sb = sp.tile([P, TC], dt.float32, tag="yo")
                    nc.vector.tensor_mul(y_sb[:, :tn], y_ps[:, :tn], tw[:, tsl])
                    nc.sync.dma_start(out=y_d[do, :, tsl], in_=y_sb[:, :tn])

    nc.compile()
    return nc


def _tile_w_in(w_t):
    """[D, FF] (already transposed) -> [FF/P, P, D/P, P] contiguous bf16."""
    # out[f, p, ki, m] = w_t[ki*128+p, f*128+m]
    r = w_t.reshape(KT, P, FT, P).transpose(2, 1, 0, 3)
    return np.ascontiguousarray(r, dtype=ml_dtypes.bfloat16)


def _tile_w_down(w):
    """w_down [D, FF] -> [D/P, P, FF/P, P] contiguous bf16.

    out[do, p, f, m] = w[do*128+m, f*128+p]
    """
    r = w.reshape(DT, P, FT, P).transpose(0, 3, 2, 1)
    return np.ascontiguousarray(r, dtype=ml_dtypes.bfloat16)


def kernel(hidden_states, gate_w, w_gate, w_up, w_down):
    from concourse.bass_utils import run_bass_kernel_spmd

    hidden_states = np.asarray(hidden_states)
    gate_w = np.asarray(gate_w)
    w_gate = np.asarray(w_gate)
    w_up = np.asarray(w_up)
    w_down = np.asarray(w_down)

    x = hidden_states.reshape(T, D)

    # --- router (tiny: T x E) on host, fp64 for stable argmax ---
    logits = x.astype(np.float64) @ gate_w.astype(np.float64).T  # [T, E]
    m = logits.max(axis=1, keepdims=True)
    p = np.exp(logits - m)
    p /= p.sum(axis=1, keepdims=True)
    sel = np.argmax(p, axis=1)  # [T]
    top_w = p[np.arange(T), sel].astype(np.float32)  # [T]

    # --- dispatch: split each expert's tokens across its cores ---
    idx_e = [np.nonzero(sel == e)[0] for e in range(E)]
    t0, t1 = len(idx_e[0]), len(idx_e[1])
    # choose cores per expert minimizing the max per-core load
    best = None
    for n0 in range(1, N_CORES):
        n1 = N_CORES - n0
        load = max(math.ceil(t0 / n0) if t0 else 0,
                   math.ceil(t1 / n1) if t1 else 0)
        if best is None or load < best[0]:
            best = (load, n0)
    C = max(P, ((best[0] + P - 1) // P) * P)
    n0 = best[1]
    cores_per_exp = [n0, N_CORES - n0]

    core_expert = []
    core_tok = []
    for e in range(E):
        ids = idx_e[e]
        nce = cores_per_exp[e]
        per = math.ceil(len(ids) / nce) if len(ids) else 0
        for j in range(nce):
            core_expert.append(e)
            core_tok.append(ids[j * per : (j + 1) * per])

    nc = _nc_cache.get(C)
    if nc is None:
        nc = _build_nc(C)
        _nc_cache[C] = nc

    # --- per-expert weight tiling (shared across that expert's cores) ---
    wg_tiled = [_tile_w_in(w_gate[e].T) for e in range(E)]
    wu_tiled = [_tile_w_in(w_up[e].T) for e in range(E)]
    wd_tiled = [_tile_w_down(w_down[e]) for e in range(E)]

    in_maps = []
    for c in range(N_CORES):
        e = core_expert[c]
        ids = core_tok[c]
        n = len(ids)
        xt = np.zeros((P, KT, C), dtype=ml_dtypes.bfloat16)
        if n:
            # xc [n, D] -> [ki, p, t] -> [p, ki, t]
            xc = x[ids].astype(ml_dtypes.bfloat16)
            xt[:, :, :n] = xc.T.reshape(KT, P, n).transpose(1, 0, 2)
        tw = np.zeros((P, C), dtype=np.float32)
        if n:
            tw[:, :n] = top_w[ids][None, :]
        in_maps.append({
            "xt": xt,
            "wg": wg_tiled[e],
            "wu": wu_tiled[e],
            "wd": wd_tiled[e],
            "tw": tw,
        })

    res = run_bass_kernel_spmd(nc, in_maps, list(range(N_CORES)))
    global LAST
    LAST = res

    # --- combine ---
    out = np.zeros((T, D), dtype=np.float32)
    for c in range(N_CORES):
        ids = core_tok[c]
        n = len(ids)
        if not n:
            continue
        y = res.results[c]["y"]  # [DT, P, C]
        out[ids] = y.reshape(D, C)[:, :n].T
    return out.reshape(B, S, D)


# revision 5
# speedup vs baseline: 1.2141x; 1.2141x over previous
"""Top-1 MoE layer (Mistral MLP experts, E=2) on 8 Trainium2 cores.

Strategy (expert-parallel + data-parallel, host does dispatch/combine):
  - Host computes the tiny router (T x E logits, softmax, argmax) in fp64,
    sorts token indices by assigned expert, and splits each expert's tokens
    evenly across that expert's cores (4 cores per expert when balanced).
  - Each core receives: its packed tokens (transposed, bf16, k-tiled), its
    expert's weights pre-tiled so every device DMA is fully contiguous, and
    the routing weight per token (replicated across partitions).
  - Device kernel per core (bf16 matmuls, fp32 PSUM accumulation): FF is
    processed in quarters so each weight byte is streamed from HBM exactly
    once; h = silu(x@Wg^T) * (x@Wu^T) for a quarter stays in SBUF, partial
    down-projections accumulate into an SBUF fp32 y buffer, and the final
    quarter fuses the per-token routing-weight scale. No collectives.
  - Host scatters per-core outputs back to token order.
"""

import math

import numpy as np
import ml_dtypes

B, S, D, FF, E = 4, 2048, 2048, 8192, 2
T = B * S
P = 128
KT = D // P   # 16 contraction tiles for gate/up
FT = FF // P  # 64 f tiles
DT = D // P   # 16 output-row tiles for down
NQ = 4        # FF quarters
FQ = FT // NQ  # 16 f tiles per quarter
N_CORES = 8
MAX_N = 512   # matmul free-dim / PSUM bank limit (fp32 out)

_nc_cache: dict[int, object] = {}

# Last BassKernelResults (for external profiling harnesses).
LAST = None


def _chunks(C):
    n = max(1, math.ceil(C / MAX_N))
    tc = min(MAX_N, ((C + n - 1) // n + 7) // 8 * 8)
    sizes = []
    left = C
    for _ in range(n):
        sizes.append(min(tc, left))
        left -= sizes[-1]
    assert sum(sizes) == C and all(0 < s <= MAX_N for s in sizes)
    return sizes


def _build_nc(C: int):
    """Build + compile the single-core Bass program (SPMD across 8 cores).

    C = per-core token capacity (multiple of 8).
    """
    import concourse.mybir as mybir
    import concourse.tile as tile
    from concourse import bacc

    dt = mybir.dt
    nc = bacc.Bacc("TRN2", target_bir_lowering=False, debug=False,
                   num_devices=N_CORES)

    # xt[p, ki, t] = x_packed[t, ki*128 + p]
    xt_d = nc.dram_tensor("xt", [P, KT, C], dt.bfloat16, kind="ExternalInput")
    # wg[f, p, ki, m] = w_gate[f*128+m, ki*128+p] (one expert)
    wg_d = nc.dram_tensor("wg", [FT, P, KT, P], dt.bfloat16, kind="ExternalInput")
    wu_d = nc.dram_tensor("wu", [FT, P, KT, P], dt.bfloat16, kind="ExternalInput")
    # wd[do, q, p, fl, m] = w_down[do*128+m, (q*FQ+fl)*128+p]
    wd_d = nc.dram_tensor("wd", [DT, NQ, P, FQ, P], dt.bfloat16,
                          kind="ExternalInput")
    # tw[p, t] = routing weight of token t (same for all p)
    tw_d = nc.dram_tensor("tw", [P, C], dt.float32, kind="ExternalInput")
    # y[do, m, t] = out_packed[t, do*128+m]
    y_d = nc.dram_tensor("y", [DT, P, C], dt.float32, kind="ExternalOutput")

    sizes = _chunks(C)
    starts = [sum(sizes[:i]) for i in range(len(sizes))]
    TC = sizes[0]

    with tile.TileContext(nc) as tc:
        with (
            tc.tile_pool(name="persist", bufs=1) as pp,
            tc.tile_pool(name="wgwu", bufs=3) as wp,
            tc.tile_pool(name="wdp", bufs=2) as dp,
            tc.tile_pool(name="hbuf", bufs=1) as hp,
            tc.tile_pool(name="stage", bufs=2) as sp,
            tc.tile_pool(name="psum", bufs=2, space="PSUM") as psp,
        ):
            xt = pp.tile([P, KT, C], dt.bfloat16)
            for ki in range(KT):
                nc.sync.dma_start(out=xt[:, ki : ki + 1, :],
                                  in_=xt_d[:, ki : ki + 1, :])
            tw = pp.tile([P, C], dt.float32)
            nc.sync.dma_start(out=tw[:], in_=tw_d[:])
            h = hp.tile([P, FQ, C], dt.bfloat16)
            y_acc = pp.tile([P, DT, C], dt.float32)

            for q in range(NQ):
                # phase A: h[fl] = silu(x @ Wg^T) * (x @ Wu^T) for this quarter
                for fl in range(FQ):
                    f = q * FQ + fl
                    wg_t = wp.tile([P, KT, P], dt.bfloat16, tag="wg")
                    nc.sync.dma_start(out=wg_t[:], in_=wg_d[f])
                    wu_t = wp.tile([P, KT, P], dt.bfloat16, tag="wu")
                    nc.sync.dma_start(out=wu_t[:], in_=wu_d[f])
                    for c, (t0, tn) in enumerate(zip(starts, sizes)):
                        tsl = slice(t0, t0 + tn)
                        g_ps = psp.tile([P, TC], dt.float32, tag="g")
                        u_ps = psp.tile([P, TC], dt.float32, tag="u")
                        for ki in range(KT):
                            nc.tensor.matmul(
                                g_ps[:, :tn],
                                wg_t[:, ki : ki + 1, :],
                                xt[:, ki : ki + 1, tsl],
                                start=(ki == 0),
                                stop=(ki == KT - 1),
                            )
                        for ki in range(KT):
                            nc.tensor.matmul(
                                u_ps[:, :tn],
                                wu_t[:, ki : ki + 1, :],
                                xt[:, ki : ki + 1, tsl],
                                start=(ki == 0),
                                stop=(ki == KT - 1),
                            )
                        sg = sp.tile([P, TC], dt.float32, tag="sg")
                        nc.scalar.activation(
                            sg[:, :tn], g_ps[:, :tn],
                            mybir.ActivationFunctionType.Silu,
                        )
                        nc.vector.tensor_mul(
                            h[:, fl, tsl], sg[:, :tn], u_ps[:, :tn]
                        )
                # phase B: y_acc += h @ Wd^T (this quarter's partial)
                for do in range(DT):
                    wd_t = dp.tile([P, FQ, P], dt.bfloat16, tag="wd")
                    nc.sync.dma_start(out=wd_t[:], in_=wd_d[do, q])
                    for c, (t0, tn) in enumerate(zip(starts, sizes)):
                        tsl = slice(t0, t0 + tn)
                        y_ps = psp.tile([P, TC], dt.float32, tag="y")
                        for fl in range(FQ):
                            nc.tensor.matmul(
                                y_ps[:, :tn],
                                wd_t[:, fl : fl + 1, :],
                                h[:, fl : fl + 1, tsl],
                                start=(fl == 0),
                                stop=(fl == FQ - 1),
                            )
                        if q == 0:
                            nc.vector.tensor_copy(
                                y_acc[:, do, tsl], y_ps[:, :tn]
                            )
                        else:
                            nc.vector.tensor_add(
                                y_acc[:, do, tsl], y_acc[:, do, tsl],
                                y_ps[:, :tn],
                            )
                        if q == NQ - 1:
                            y_sb = sp.tile([P, TC], dt.float32, tag="yo")
                            nc.vector.tensor_mul(
                                y_sb[:, :tn], y_acc[:, do, tsl], tw[:, tsl]
                            )
                            nc.sync.dma_start(
                                out=y_d[do, :, tsl], in_=y_sb[:, :tn]
                            )

    nc.compile()
    return nc


def _tile_w_in(w_t):
    """[D, FF] (already transposed) -> [FF/P, P, D/P, P] contiguous bf16."""
    # out[f, p, ki, m] = w_t[ki*128+p, f*128+m]
    r = w_t.reshape(KT, P, FT, P).transpose(2, 1, 0, 3)
    return np.ascontiguousarray(r, dtype=ml_dtypes.bfloat16)


def _tile_w_down(w):
    """w_down [D, FF] -> [D/P, NQ, P, FQ, P] contiguous bf16.

    out[do, q, p, fl, m] = w[do*128+m, (q*FQ+fl)*128+p]
    """
    r = w.reshape(DT, P, NQ, FQ, P).transpose(0, 2, 4, 3, 1)
    return np.ascontiguousarray(r, dtype=ml_dtypes.bfloat16)


def kernel(hidden_states, gate_w, w_gate, w_up, w_down):
    from concourse.bass_utils import run_bass_kernel_spmd

    hidden_states = np.asarray(hidden_states)
    gate_w = np.asarray(gate_w)
    w_gate = np.asarray(w_gate)
    w_up = np.asarray(w_up)
    w_down = np.asarray(w_down)

    x = hidden_states.reshape(T, D)

    # --- router (tiny: T x E) on host, fp64 for stable argmax ---
    logits = x.astype(np.float64) @ gate_w.astype(np.float64).T  # [T, E]
    m = logits.max(axis=1, keepdims=True)
    p = np.exp(logits - m)
    p /= p.sum(axis=1, keepdims=True)
    sel = np.argmax(p, axis=1)  # [T]
    top_w = p[np.arange(T), sel].astype(np.float32)  # [T]

    # --- dispatch: split each expert's tokens across its cores ---
    idx_e = [np.nonzero(sel == e)[0] for e in range(E)]
    t0, t1 = len(idx_e[0]), len(idx_e[1])
    # choose cores per expert minimizing the max per-core load
    best = None
    for n0 in range(1, N_CORES):
        n1 = N_CORES - n0
        load = max(math.ceil(t0 / n0) if t0 else 0,
                   math.ceil(t1 / n1) if t1 else 0)
        if best is None or load < best[0]:
            best = (load, n0)
    # pad capacity to a multiple of 8; matmul/DVE free dims and DMA shapes
    # handle arbitrary sizes, so no 128-rounding.
    C = max(P, ((best[0] + 7) // 8) * 8)
    n0 = best[1]
    cores_per_exp = [n0, N_CORES - n0]

    core_expert = []
    core_tok = []
    for e in range(E):
        ids = idx_e[e]
        nce = cores_per_exp[e]
        per = math.ceil(len(ids) / nce) if len(ids) else 0
        for j in range(nce):
            core_expert.append(e)
            core_tok.append(ids[j * per : (j + 1) * per])

    nc = _nc_cache.get(C)
    if nc is None:
        nc = _build_nc(C)
        _nc_cache[C] = nc

    # --- per-expert weight tiling (shared across that expert's cores) ---
    wg_tiled = [_tile_w_in(w_gate[e].T) for e in range(E)]
    wu_tiled = [_tile_w_in(w_up[e].T) for e in range(E)]
    wd_tiled = [_tile_w_down(w_down[e]) for e in range(E)]

    in_maps = []
    for c in range(N_CORES):
        e = core_expert[c]
        ids = core_tok[c]
        n = len(ids)
        xt = np.zeros((P, KT, C), dtype=ml_dtypes.bfloat16)
        if n:
            # xc [n, D] -> [ki, p, t] -> [p, ki, t]
            xc = x[ids].astype(ml_dtypes.bfloat16)
            xt[:, :, :n] = xc.T.reshape(KT, P, n).transpose(1, 0, 2)
        tw = np.zeros((P, C), dtype=np.float32)
        if n:
            tw[:, :n] = top_w[ids][None, :]
        in_maps.append({
            "xt": xt,
            "wg": wg_tiled[e],
            "wu": wu_tiled[e],
            "wd": wd_tiled[e],
            "tw": tw,
        })

    res = run_bass_kernel_spmd(nc, in_maps, list(range(N_CORES)))
    global LAST
    LAST = res

    # --- combine ---
    out = np.zeros((T, D), dtype=np.float32)
    for c in range(N_CORES):
        ids = core_tok[c]
        n = len(ids)
        if not n:
            continue
        y = res.results[c]["y"]  # [DT, P, C]
        out[ids] = y.reshape(D, C)[:, :n].T
    return out.reshape(B, S, D)


# revision 9
# speedup vs baseline: 1.2157x; 1.0013x over previous
"""Top-1 MoE layer (Mistral MLP experts, E=2) on 8 Trainium2 cores.

Strategy (expert-parallel + data-parallel, host does dispatch/combine):
  - Host computes the tiny router (T x E logits, softmax, argmax) in fp64,
    sorts token indices by assigned expert, and splits each expert's tokens
    evenly across that expert's cores (4 cores per expert when balanced).
  - Each core receives: its packed tokens (transposed, bf16, k-tiled), its
    expert's weights pre-tiled so every device DMA is fully contiguous, and
    the routing weight per token (replicated across partitions).
  - Device kernel per core (bf16 matmuls, fp32 PSUM accumulation): FF is
    processed in quarters so each weight byte is streamed from HBM exactly
    once; h = silu(x@Wg^T) * (x@Wu^T) for a quarter stays in SBUF, partial
    down-projections accumulate into an SBUF fp32 y buffer, and the final
    quarter fuses the per-token routing-weight scale. No collectives.
  - Host scatters per-core outputs back to token order.
"""

import math

import numpy as np
import ml_dtypes

B, S, D, FF, E = 4, 2048, 2048, 8192, 2
T = B * S
P = 128
KT = D // P   # 16 contraction tiles for gate/up
FT = FF // P  # 64 f tiles
DT = D // P   # 16 output-row tiles for down
NQ = 4        # FF quarters
FQ = FT // NQ  # 16 f tiles per quarter
N_CORES = 8
MAX_N = 512   # matmul free-dim / PSUM bank limit (fp32 out)

_nc_cache: dict[int, object] = {}

# Last BassKernelResults (for external profiling harnesses).
LAST = None


def _chunks(C):
    n = max(1, math.ceil(C / MAX_N))
    tc = min(MAX_N, ((C + n - 1) // n + 7) // 8 * 8)
    sizes = []
    left = C
    for _ in range(n):
        sizes.append(min(tc, left))
        left -= sizes[-1]
    assert sum(sizes) == C and all(0 < s <= MAX_N for s in sizes)
    return sizes


def _build_nc(C: int):
    """Build + compile the single-core Bass program (SPMD across 8 cores).

    C = per-core token capacity (multiple of 8).
    """
    import concourse.mybir as mybir
    import concourse.tile as tile
    from concourse import bacc

    dt = mybir.dt
    nc = bacc.Bacc("TRN2", target_bir_lowering=False, debug=False,
                   num_devices=N_CORES)

    # xt[p, ki, t] = x_packed[t, ki*128 + p]
    xt_d = nc.dram_tensor("xt", [P, KT, C], dt.bfloat16, kind="ExternalInput")
    # wg[f, p, ki, m] = w_gate[f*128+m, ki*128+p] (one expert)
    wg_d = nc.dram_tensor("wg", [FT, P, KT, P], dt.bfloat16, kind="ExternalInput")
    wu_d = nc.dram_tensor("wu", [FT, P, KT, P], dt.bfloat16, kind="ExternalInput")
    # wd[do, q, p, fl, m] = w_down[do*128+m, (q*FQ+fl)*128+p]
    wd_d = nc.dram_tensor("wd", [DT, NQ, P, FQ, P], dt.bfloat16,
                          kind="ExternalInput")
    # tw[p, t] = routing weight of token t (same for all p)
    tw_d = nc.dram_tensor("tw", [P, C], dt.float32, kind="ExternalInput")
    # y[do, m, t] = out_packed[t, do*128+m]
    y_d = nc.dram_tensor("y", [DT, P, C], dt.float32, kind="ExternalOutput")

    sizes = _chunks(C)
    starts = [sum(sizes[:i]) for i in range(len(sizes))]
    TC = sizes[0]
    # at very large C (heavily skewed routing) the resident x/h/y buffers
    # leave less SBUF headroom — shrink the weight-stream double-buffering
    wbufs = 3 if C <= 1100 else 2

    with tile.TileContext(nc) as tc:
        with (
            tc.tile_pool(name="persist", bufs=1) as pp,
            tc.tile_pool(name="wgwu", bufs=wbufs) as wp,
            tc.tile_pool(name="wdp", bufs=2) as dp,
            tc.tile_pool(name="hbuf", bufs=1) as hp,
            tc.tile_pool(name="stage", bufs=2) as sp,
            tc.tile_pool(name="psum", bufs=2, space="PSUM") as psp,
        ):
            xt = pp.tile([P, KT, C], dt.bfloat16)
            # Load chunk 0 of x on the sync HWDGE queue (ahead of the weight
            # stream, FIFO) so the first matmul group starts ASAP; later
            # chunks + tw go on the scalar HWDGE queue to stay off the
            # critical path.
            for c, (t0, tn) in enumerate(zip(starts, sizes)):
                eng = nc.sync if c == 0 else nc.scalar
                eng.dma_start(
                    out=xt[:, :, t0 : t0 + tn],
                    in_=xt_d[:, :, t0 : t0 + tn],
                )
            tw = pp.tile([P, C], dt.float32)
            nc.scalar.dma_start(out=tw[:], in_=tw_d[:])
            h = hp.tile([P, FQ, C], dt.bfloat16)
            y_acc = pp.tile([P, DT, C], dt.float32)

            for q in range(NQ):
                # phase A: h[fl] = silu(x @ Wg^T) * (x @ Wu^T) for this quarter
                for fl in range(FQ):
                    f = q * FQ + fl
                    wg_t = wp.tile([P, KT, P], dt.bfloat16, tag="wg")
                    nc.sync.dma_start(out=wg_t[:], in_=wg_d[f])
                    wu_t = wp.tile([P, KT, P], dt.bfloat16, tag="wu")
                    nc.sync.dma_start(out=wu_t[:], in_=wu_d[f])
                    for c, (t0, tn) in enumerate(zip(starts, sizes)):
                        tsl = slice(t0, t0 + tn)
                        g_ps = psp.tile([P, TC], dt.float32, tag="g")
                        u_ps = psp.tile([P, TC], dt.float32, tag="u")
                        for ki in range(KT):
                            nc.tensor.matmul(
                                g_ps[:, :tn],
                                wg_t[:, ki : ki + 1, :],
                                xt[:, ki : ki + 1, tsl],
                                start=(ki == 0),
                                stop=(ki == KT - 1),
                            )
                        for ki in range(KT):
                            nc.tensor.matmul(
                                u_ps[:, :tn],
                                wu_t[:, ki : ki + 1, :],
                                xt[:, ki : ki + 1, tsl],
                                start=(ki == 0),
                                stop=(ki == KT - 1),
                            )
                        sg = sp.tile([P, TC], dt.float32, tag="sg")
                        nc.scalar.activation(
                            sg[:, :tn], g_ps[:, :tn],
                            mybir.ActivationFunctionType.Silu,
                        )
                        nc.vector.tensor_mul(
                            h[:, fl, tsl], sg[:, :tn], u_ps[:, :tn]
                        )
                # phase B: y_acc += h @ Wd^T (this quarter's partial)
                for do in range(DT):
                    wd_t = dp.tile([P, FQ, P], dt.bfloat16, tag="wd")
                    nc.sync.dma_start(out=wd_t[:], in_=wd_d[do, q])
                    for c, (t0, tn) in enumerate(zip(starts, sizes)):
                        tsl = slice(t0, t0 + tn)
                        y_ps = psp.tile([P, TC], dt.float32, tag="y")
                        for fl in range(FQ):
                            nc.tensor.matmul(
                                y_ps[:, :tn],
                                wd_t[:, fl : fl + 1, :],
                                h[:, fl : fl + 1, tsl],
                                start=(fl == 0),
                                stop=(fl == FQ - 1),
                            )
                        if q == 0:
                            nc.vector.tensor_copy(
                                y_acc[:, do, tsl], y_ps[:, :tn]
                            )
                        else:
                            nc.vector.tensor_add(
                                y_acc[:, do, tsl], y_acc[:, do, tsl],
                                y_ps[:, :tn],
                            )
                        if q == NQ - 1:
                            y_sb = sp.tile([P, TC], dt.float32, tag="yo")
                            nc.vector.tensor_mul(
                                y_sb[:, :tn], y_acc[:, do, tsl], tw[:, tsl]
                            )
                            nc.sync.dma_start(
                                out=y_d[do, :, tsl], in_=y_sb[:, :tn]
                            )

    nc.compile()
    return nc


def _tile_w_in(w_t):
    """[D, FF] (already transposed) -> [FF/P, P, D/P, P] contiguous bf16."""
    # out[f, p, ki, m] = w_t[ki*128+p, f*128+m]
    r = w_t.reshape(KT, P, FT, P).transpose(2, 1, 0, 3)
    return np.ascontiguousarray(r, dtype=ml_dtypes.bfloat16)


def _tile_w_down(w):
    """w_down [D, FF] -> [D/P, NQ, P, FQ, P] contiguous bf16.

    out[do, q, p, fl, m] = w[do*128+m, (q*FQ+fl)*128+p]
    """
    r = w.reshape(DT, P, NQ, FQ, P).transpose(0, 2, 4, 3, 1)
    return np.ascontiguousarray(r, dtype=ml_dtypes.bfloat16)


def kernel(hidden_states, gate_w, w_gate, w_up, w_down):
    from concourse.bass_utils import run_bass_kernel_spmd

    hidden_states = np.asarray(hidden_states)
    gate_w = np.asarray(gate_w)
    w_gate = np.asarray(w_gate)
    w_up = np.asarray(w_up)
    w_down = np.asarray(w_down)

    x = hidden_states.reshape(T, D)

    # --- router (tiny: T x E) on host, fp64 for stable argmax ---
    logits = x.astype(np.float64) @ gate_w.astype(np.float64).T  # [T, E]
    m = logits.max(axis=1, keepdims=True)
    p = np.exp(logits - m)
    p /= p.sum(axis=1, keepdims=True)
    sel = np.argmax(p, axis=1)  # [T]
    top_w = p[np.arange(T), sel].astype(np.float32)  # [T]

    # --- dispatch: split each expert's tokens across its cores ---
    idx_e = [np.nonzero(sel == e)[0] for e in range(E)]
    t0, t1 = len(idx_e[0]), len(idx_e[1])
    # choose cores per expert minimizing the max per-core load
    best = None
    for n0 in range(1, N_CORES):
        n1 = N_CORES - n0
        load = max(math.ceil(t0 / n0) if t0 else 0,
                   math.ceil(t1 / n1) if t1 else 0)
        if best is None or load < best[0]:
            best = (load, n0)
    # pad capacity to a multiple of 8; matmul/DVE free dims and DMA shapes
    # handle arbitrary sizes, so no 128-rounding.
    C = max(P, ((best[0] + 7) // 8) * 8)
    n0 = best[1]
    cores_per_exp = [n0, N_CORES - n0]

    core_expert = []
    core_tok = []
    for e in range(E):
        ids = idx_e[e]
        nce = cores_per_exp[e]
        per = math.ceil(len(ids) / nce) if len(ids) else 0
        for j in range(nce):
            core_expert.append(e)
            core_tok.append(ids[j * per : (j + 1) * per])

    nc = _nc_cache.get(C)
    if nc is None:
        nc = _build_nc(C)
        _nc_cache[C] = nc

    # --- per-expert weight tiling (shared across that expert's cores) ---
    wg_tiled = [_tile_w_in(w_gate[e].T) for e in range(E)]
    wu_tiled = [_tile_w_in(w_up[e].T) for e in range(E)]
    wd_tiled = [_tile_w_down(w_down[e]) for e in range(E)]

    in_maps = []
    for c in range(N_CORES):
        e = core_expert[c]
        ids = core_tok[c]
        n = len(ids)
        xt = np.zeros((P, KT, C), dtype=ml_dtypes.bfloat16)
        if n:
            # xc [n, D] -> [ki, p, t] -> [p, ki, t]
            xc = x[ids].astype(ml_dtypes.bfloat16)
            xt[:, :, :n] = xc.T.reshape(KT, P, n).transpose(1, 0, 2)
        tw = np.zeros((P, C), dtype=np.float32)
        if n:
            tw[:, :n] = top_w[ids][None, :]
        in_maps.append({
            "xt": xt,
            "wg": wg_tiled[e],
            "wu": wu_tiled[e],
            "wd": wd_tiled[e],
            "tw": tw,
        })

    res = run_bass_kernel_spmd(nc, in_maps, list(range(N_CORES)))
    global LAST
    LAST = res

    # --- combine ---
    out = np.zeros((T, D), dtype=np.float32)
    for c in range(N_CORES):
        ids = core_tok[c]
        n = len(ids)
        if not n:
            continue
        y = res.results[c]["y"]  # [DT, P, C]
        out[ids] = y.reshape(D, C)[:, :n].T
    return out.reshape(B, S, D)
